# revision 1
# baseline (speedup 1.0000x reference)
"""AGNO block (edge-softmax GNN message passing) on 8 TRN2 NeuronCores.

Strategy (per sharding hint): partition edges by target across the 8 cores
(equal 2500-target ranges, edge ranges via searchsorted on the sorted
trg_idx).  Node features + small weights are replicated.

v2: the per-edge gather traffic is collapsed to ONE dma_gather per block
(256B/edge) by algebraically folding everything except [f_x | x] out of
the per-edge data:

  - scores:  Q_t . K_s / 8 = x_s . v_t + c_t   with  v = pe_W (Wk Q/8),
    c = pe_b.(Wk Q/8) + bk.Q/8 - both per-target [4, T], SBUF-resident.
    Per 128-edge chunk: S[e, t] = matmul(lhsT=x-rows of G, rhs=V-block),
    then mask by the one-hot target matrix and row-reduce -> per-edge s.
  - y_e for the MLP is injected per tile via A_blk = ye_blk^T @ W1_y and a
    matmul against onehot[t, e]; the x_e part of layer 1 is folded into W1
    (pe_W @ W1_x rows + const row).
  - The per-edge src table is [f_emb(64) | x(3) | 1 | 0(60)], 128ch bf16 =
    256B/edge, one transpose-gather per block.
  - Both one-hot layouts are built on device from two tiny per-edge arrays
    (relrow broadcast via SBUF DMA + is_equal; relb/iotab broadcast APs),
    keeping the host->device input volume at ~5MB/core.

  per 128-target block (20 blocks x 7 tiles x 512 edges):
    scores for all tiles -> Taylor exp(s) on DVE -> per tile: 3-layer MLP
    channel-major on PE (bf16, f32 psum, fused gelu+bias), agg =
    (k_out + b3) * f_emb[src], PE transpose, one-hot scatter-matmul
    accumulating psum[65,128] = [sum z*agg | sum z], then normalize + DMA.

All inputs ship as two packed blob tensors (per-call dispatch cost scales
with the number and size of input buffers under the PJRT path).

The output rows [2500:2560) of each core are padding and discarded.
"""

import ml_dtypes
import numpy as np

import concourse.bass as bass
import concourse.tile as tile
from concourse import bacc, mybir
from concourse.bass_utils import run_bass_kernel_spmd

F32 = mybir.dt.float32
BF16 = mybir.dt.bfloat16
I16 = mybir.dt.int16
AF = mybir.ActivationFunctionType
ALU = mybir.AluOpType
AX = mybir.AxisListType


def make_cfg(n_nodes=20000, tpc=2500, n_cores=8, ts=512, tpb=7, nblk=20):
    g = {}
    g["NC"] = n_cores
    g["N"] = n_nodes
    g["NPAD"] = -(-n_nodes // 512) * 512          # node pad to 512
    g["G"] = g["NPAD"] // 128
    g["TPC"] = tpc                                 # real targets per core
    g["NBLK"] = nblk                               # 128-target blocks per core
    g["TPAD"] = nblk * 128
    g["GT"] = g["TPAD"] // 128
    g["TS"] = ts                                   # edges per tile
    g["CH"] = ts // 128
    g["TPB"] = tpb                                 # tiles per block (padded)
    g["EPAD"] = nblk * tpb * ts
    assert g["TPAD"] >= tpc
    assert g["NPAD"] < 2 ** 15 and g["TPAD"] < 2 ** 15  # int16 gather idx
    return g


CFG_FULL = make_cfg()


def input_layouts(cfg):
    """Canonical packing order for the two blob inputs (host & device)."""
    NPAD, TPAD, EPAD = cfg["NPAD"], cfg["TPAD"], cfg["EPAD"]
    f32 = [
        ("peB", (32, 1)), ("bq", (64, 1)), ("liftB", (64, 1)),
        ("b1a", (128, 1)), ("b1b", (128, 1)), ("b2c", (128, 1)),
        ("b3c", (64, 1)),
        ("iotac", (128, 1)),
    ]
    bf16 = [
        ("peW", (3, 32)), ("Wq", (32, 64)),
        ("WkT8", (64, 32)), ("peWT", (32, 32)),
        ("peb32", (32, 1)), ("bk8", (64, 1)),
        ("liftW", (64, 64)),
        ("W1a", (128, 128)), ("W1b", (128, 128)), ("W1y", (32, 256)),
        ("W2a", (128, 128)), ("W2b", (128, 128)), ("W3w", (128, 64)),
        ("x4", (4, NPAD)), ("fcm", (64, NPAD)), ("ycm", (3, TPAD)),
        ("srcw16", (16, EPAD // 16)),
        ("relrow", (1, EPAD)), ("relb", (128, EPAD // 128)),
        ("iotab", (128, 128)),
    ]
    return f32, bf16


def _pack_blob(layout, arrays, np_dtype):
    chunks = []
    for name, shape in layout:
        a = np.asarray(arrays[name])
        assert tuple(a.shape) == tuple(shape), (name, a.shape, shape)
        flat = a.ravel()
        pad = (-len(flat)) % 64
        if pad:
            flat = np.concatenate([flat, np.zeros(pad, a.dtype)])
        chunks.append(flat)
    return np.ascontiguousarray(np.concatenate(chunks)).astype(np_dtype).reshape(1, -1)


def _carve_blob(blob_ap, layout):
    out = {}
    off = 0
    for name, shape in layout:
        n = int(np.prod(shape))
        ap = blob_ap[0:1, off:off + n].rearrange("a (p c) -> (a p) c", p=shape[0])
        out[name] = ap
        off += n + ((-n) % 64)
    return out, off


def _wrap16(a):
    # idx j -> (partition j%16, col j//16), replicated to 128 partitions
    w = a.reshape(-1, 16).T
    return np.tile(w, (8, 1)).copy()


def host_prep(inputs, cfg):
    """Split + relayout inputs; returns per-core input maps."""
    N, NPAD = cfg["N"], cfg["NPAD"]
    TPC, TPAD = cfg["TPC"], cfg["TPAD"]
    NBLK, TS, TPB, EPAD = cfg["NBLK"], cfg["TS"], cfg["TPB"], cfg["EPAD"]

    x = np.asarray(inputs["x"], np.float32)
    y = np.asarray(inputs["y"], np.float32)
    f_x = np.asarray(inputs["f_x"], np.float32)
    src = np.asarray(inputs["src_idx"], np.int64)
    trg = np.asarray(inputs["trg_idx"], np.int64)

    # node permutation so that table row r holds node r after dma-transpose
    jj = np.arange(NPAD)
    perm_n = (jj % 128) * (NPAD // 128) + jj // 128

    xpad = np.zeros((NPAD, 3), np.float32); xpad[:N] = x
    fpad = np.zeros((NPAD, 64), np.float32); fpad[:N] = f_x
    x4 = np.concatenate([xpad[perm_n].T, np.ones((1, NPAD), np.float32)])
    fcm = np.ascontiguousarray(fpad[perm_n].T).astype(ml_dtypes.bfloat16)

    pe_W = np.asarray(inputs["pe_W"], np.float32)    # [3, 32]
    pe_b = np.asarray(inputs["pe_b"], np.float32)    # [32]
    Wk = np.asarray(inputs["Wk"], np.float32)        # [32, 64]
    bk = np.asarray(inputs["bk"], np.float32)        # [64]
    W1 = np.asarray(inputs["W1"], np.float32)        # [128, 256]
    # device G rows: [f_emb(0:64) | x(64:67) | 1(67) | 0(68:128)]; the y_e
    # part of layer 1 is injected per tile via A_blk = ye_blk^T @ W1_y.
    W1p = np.zeros((128, 256), np.float32)
    W1p[0:64] = W1[64:128]                           # f part
    W1p[64:67] = pe_W @ W1[32:64]                    # x part folded thru pe
    W1p[67] = pe_b @ W1[32:64]                       # pe_b const fold
    b1 = np.asarray(inputs["b1"], np.float32)
    W2 = np.asarray(inputs["W2"], np.float32)        # [256, 128]
    b2 = np.asarray(inputs["b2"], np.float32)
    W3 = np.asarray(inputs["W3"], np.float32)        # [128, 64]
    b3 = np.asarray(inputs["b3"], np.float32)

    shared = {
        "x4": np.ascontiguousarray(x4).astype(ml_dtypes.bfloat16), "fcm": fcm,
        "peW": pe_W,
        "peB": pe_b.reshape(32, 1).copy(),
        "Wq": np.asarray(inputs["Wq"], np.float32),
        "bq": np.asarray(inputs["bq"], np.float32).reshape(64, 1),
        "WkT8": np.ascontiguousarray(Wk.T) / 8.0,
        "peWT": np.pad(np.ascontiguousarray(pe_W.T), ((0, 0), (0, 29))),
        "peb32": pe_b.reshape(32, 1).copy(),
        "bk8": (bk / 8.0).reshape(64, 1),
        "liftW": np.asarray(inputs["lift_W"], np.float32),
        "liftB": np.asarray(inputs["lift_b"], np.float32).reshape(64, 1),
        "W1a": np.ascontiguousarray(W1p[:, 0:128]),
        "W1b": np.ascontiguousarray(W1p[:, 128:256]),
        "W1y": np.ascontiguousarray(W1[0:32]),

        "b1a": b1[0:128].reshape(128, 1).copy(),
        "b1b": b1[128:256].reshape(128, 1).copy(),
        "W2a": np.ascontiguousarray(W2[0:128]),
        "W2b": np.ascontiguousarray(W2[128:256]),
        "b2c": b2.reshape(128, 1).copy(),
        "W3w": W3,
        "b3c": b3.reshape(64, 1).copy(),
        "iotac": np.arange(128, dtype=np.float32).reshape(128, 1),
        "iotab": np.broadcast_to(np.arange(128), (128, 128)).astype(
            ml_dtypes.bfloat16).copy(),
    }

    in_maps = []
    for c in range(cfg["NC"]):
        t0 = c * TPC
        ypad = np.zeros((TPAD, 3), np.float32)
        ypad[:TPC] = y[t0:t0 + TPC]
        ycm = np.ascontiguousarray(ypad.T).astype(ml_dtypes.bfloat16)

        srcpad = np.zeros(EPAD, np.int16)
        trgrel = np.full(EPAD, -1.0, np.float32)
        for b in range(NBLK):
            lo = t0 + b * 128
            hi = min(lo + 128, t0 + TPC)
            e0 = np.searchsorted(trg, lo, side="left")
            e1 = np.searchsorted(trg, hi, side="left")
            n = e1 - e0
            base = b * TPB * TS
            assert n <= TPB * TS, f"core {c} block {b}: {n} edges > {TPB*TS}"
            srcpad[base:base + n] = src[e0:e1]
            trgrel[base:base + n] = (trg[e0:e1] - lo).astype(np.float32)

        m = dict(shared)
        m["ycm"] = ycm
        m["srcw16"] = srcpad.reshape(-1, 16).T.copy().view(
            ml_dtypes.bfloat16)                              # [16, EPAD//16]
        m["relrow"] = trgrel.reshape(1, EPAD).astype(ml_dtypes.bfloat16)
        m["relb"] = np.ascontiguousarray(
            trgrel.reshape(EPAD // 128, 128).T).astype(ml_dtypes.bfloat16)
        lay32, lay16 = input_layouts(cfg)
        in_maps.append({
            "cblob": _pack_blob(lay32, m, np.float32),
            "dblob": _pack_blob(lay16, m, ml_dtypes.bfloat16),
        })
    return in_maps


def build(tc, out_ap, ins, cfg):
    """Emit the per-core graph. ins: dict name -> DRAM AP."""
    nc = tc.nc
    N, NPAD = cfg["N"], cfg["NPAD"]
    TPAD = cfg["TPAD"]
    NBLK, TS, CH, TPB, EPAD = cfg["NBLK"], cfg["TS"], cfg["CH"], cfg["TPB"], cfg["EPAD"]
    BTS = TPB * TS                                   # edges per block

    tsrc = nc.dram_tensor("tsrc_tab", [NPAD, 128], BF16)

    const = tc.alloc_tile_pool(name="const", bufs=1)
    resid = tc.alloc_tile_pool(name="resid", bufs=1)

    def load(name, shape, dtype=F32):
        t = const.tile(shape, dtype, tag=name)
        nc.sync.dma_start(t[:], ins[name])
        return t

    def load16(name, shape):
        t = const.tile(shape, BF16, tag=name)
        nc.sync.dma_start(t[:], ins[name])
        return t

    peB = load("peB", [32, 1]); bq = load("bq", [64, 1])
    liftB = load("liftB", [64, 1])
    b1a = load("b1a", [128, 1]); b1b = load("b1b", [128, 1])
    b2c = load("b2c", [128, 1]); b3c = load("b3c", [64, 1])
    iotac = load("iotac", [128, 1])
    iotab = load16("iotab", [128, 128])
    ident = const.tile([128, 128], F32, tag="ident")
    nc.vector.tensor_scalar(ident[:], iotab[:], iotac[:], None,
                            ALU.is_equal)
    peW_bf = load16("peW", [3, 32])
    Wq_bf = load16("Wq", [32, 64])
    WkT8_bf = load16("WkT8", [64, 32])
    peWT_bf = load16("peWT", [32, 32])
    peb32_bf = load16("peb32", [32, 1])
    bk8_bf = load16("bk8", [64, 1])
    liftW_bf = load16("liftW", [64, 64])
    W1a_bf = load16("W1a", [128, 128])
    W1b_bf = load16("W1b", [128, 128])
    W1y_bf = load16("W1y", [32, 256])
    W2a_bf = load16("W2a", [128, 128])
    W2b_bf = load16("W2b", [128, 128])
    W3_bf = load16("W3w", [128, 64])
    id_bf = const.tile([128, 128], BF16, tag="id_bf")
    nc.vector.tensor_scalar(id_bf[:], iotab[:], iotac[:], None,
                            ALU.is_equal)

    # gather indices: ship one 16-partition copy, replicate to 8 groups here
    srcw = resid.tile([128, EPAD // 16], I16, tag="srcw")
    for k in range(8):
        nc.sync.dma_start(srcw[16 * k:16 * (k + 1), :], ins["srcw16"])
    relb = resid.tile([128, EPAD // 128], BF16, tag="relb")
    nc.sync.dma_start(relb[:], ins["relb"])
    # per-target score vectors [x0,x1,x2,1]-weights, at partitions 64:68
    V3 = resid.tile([3, TPAD], BF16, tag="V3")
    Vc = resid.tile([1, TPAD], BF16, tag="Vc")
    V68 = resid.tile([68, TPAD], BF16, tag="V68")
    ye_bf = resid.tile([32, TPAD], BF16, tag="ye")

    # ---------------- prep: encoders + HBM src table ----------
    HG = (NPAD // 128) // 2 if (NPAD // 128) % 2 == 0 else NPAD // 128
    HN = HG * 128                     # nodes per half
    NHALF = NPAD // HN
    with tc.tile_pool(name="cmtab", bufs=1) as cmp_, \
         tc.tile_pool(name="raw", bufs=1) as rawp, \
         tc.tile_pool(name="stage", bufs=1) as stg, \
         tc.tile_pool(name="prep_ps", bufs=2, space="PSUM") as pps, \
         tc.tile_pool(name="prep_pv", bufs=1, space="PSUM") as ppv:
        ycm = rawp.tile([3, TPAD], BF16, tag="ycm")
        nc.sync.dma_start(ycm[:], ins["ycm"])
        # target-side: y_e, Q, V = [pe_W @ (Wk Q/8); pe_b.(Wk Q/8) + bk.Q/8]
        for j0 in range(0, TPAD, TS):
            w = min(TS, TPAD - j0)
            sl = slice(j0, j0 + w)
            ps = pps.tile([32, TS], F32, tag="ps32")
            nc.tensor.matmul(ps[:, :w], peW_bf[:], ycm[:, sl])
            nc.scalar.activation(ye_bf[:, sl], ps[:, :w], AF.Identity,
                                 bias=peB[:])
            ps2 = pps.tile([64, TS], F32, tag="ps64")
            nc.tensor.matmul(ps2[:, :w], Wq_bf[:], ye_bf[:, sl])
            Qt = cmp_.tile([64, TS], BF16, tag="Qt")
            nc.vector.tensor_scalar_add(Qt[:, :w], ps2[:, :w], bq[:])
            ps3 = pps.tile([32, TS], F32, tag="ps32b")
            nc.tensor.matmul(ps3[:, :w], WkT8_bf[:], Qt[:, :w])
            ut = cmp_.tile([32, TS], BF16, tag="ut")
            nc.vector.tensor_copy(ut[:, :w], ps3[:, :w])
            psv = ppv.tile([32, TS], F32, tag="psv")
            nc.tensor.matmul(psv[:, :w], peWT_bf[:], ut[:, :w])
            nc.vector.tensor_copy(V3[:, sl], psv[0:3, :w])
            psc = ppv.tile([1, TS], F32, tag="psc")
            nc.tensor.matmul(psc[:, :w], peb32_bf[:], ut[:, :w],
                             start=True, stop=False)
            nc.tensor.matmul(psc[:, :w], bk8_bf[:], Qt[:, :w],
                             start=False, stop=True)
            nc.vector.tensor_copy(Vc[:, sl], psc[:, :w])
        # move V rows to partitions 64:68 (SBUF->SBUF DMA shifts partitions)
        nc.sync.dma_start(V68[64:67, :], V3[:])
        nc.sync.dma_start(V68[67:68, :], Vc[:])

        # src-side table halves: [f_emb(0:64) | x,1(64:68) | 0(68:96) | 0]
        tv = tsrc.ap().rearrange("(p g) c -> p g c", p=128)
        for h in range(NHALF):
            hsl = slice(h * HN, (h + 1) * HN)
            x16t = rawp.tile([16, HN], BF16, tag="x16")
            nc.vector.memset(x16t[:], 0.0)
            nc.sync.dma_start(x16t[0:4, :], ins["x4"][:, hsl])
            fcm = rawp.tile([64, HN], BF16, tag="fcm")
            nc.sync.dma_start(fcm[:], ins["fcm"][:, hsl])
            fe_bf = cmp_.tile([64, HN], BF16, tag="fe")
            for j0 in range(0, HN, TS):
                w = min(TS, HN - j0)
                sl = slice(j0, j0 + w)
                ps2 = pps.tile([64, TS], F32, tag="ps64")
                nc.tensor.matmul(ps2[:, :w], liftW_bf[:], fcm[:, sl])
                nc.vector.tensor_scalar_add(fe_bf[:, sl], ps2[:, :w], liftB[:])
            st = stg.tile([128, HG, 128], BF16, tag="stage")
            nc.vector.memset(st[:, :, 80:128], 0.0)
            nc.sync.dma_start_transpose(st[:, :, 0:64], fe_bf[:])
            nc.sync.dma_start_transpose(st[:, :, 64:80], x16t[:])
            nc.sync.dma_start(tv[:, h * HG:(h + 1) * HG, :], st[:])

    # ---------------- main loop ----------------
    with tc.tile_pool(name="gath", bufs=2) as gp, \
         tc.tile_pool(name="mask", bufs=2) as mp, \
         tc.tile_pool(name="act", bufs=2) as hp, \
         tc.tile_pool(name="small", bufs=2) as sp, \
         tc.tile_pool(name="outp", bufs=2) as op_, \
         tc.tile_pool(name="ps_sc", bufs=1, space="PSUM") as ppS, \
         tc.tile_pool(name="ps_a", bufs=1, space="PSUM") as ppA, \
         tc.tile_pool(name="ps_l1", bufs=1, space="PSUM") as pp1, \
         tc.tile_pool(name="ps_l2", bufs=1, space="PSUM") as pp2, \
         tc.tile_pool(name="ps_l3", bufs=1, space="PSUM") as pp3, \
         tc.tile_pool(name="ps_t", bufs=1, space="PSUM") as ppt, \
         tc.tile_pool(name="ps_blk", bufs=1, space="PSUM") as ppb:

        tsrc_ap = tsrc.ap()
        relrow_ap = ins["relrow"]
        for b in range(NBLK):
            i0 = b * BTS
            isl = slice(i0 // 16, (i0 + BTS) // 16)
            gb = gp.tile([128, 1, BTS], BF16, tag="g1")
            nc.gpsimd.dma_gather(gb[:], tsrc_ap[:], srcw[:, isl],
                                 num_idxs=BTS, num_idxs_reg=BTS,
                                 elem_size=128, elem_step=128,
                                 transpose=True, single_packet=False)
            # masks: replicate trgrel across partitions (broadcast-AP DMA),
            # then is_equal against the partition iota / the free-dim iota
            relrep = mp.tile([128, BTS], BF16, tag="relrep")
            rsl = relrow_ap[0:1, i0:i0 + BTS]
            rbc_dram = bass.AP(tensor=rsl.tensor, offset=rsl.offset,
                               ap=[[0, 128], rsl.ap[-1]])
            nc.sync.dma_start(relrep[:], rbc_dram)
            ohte = mp.tile([128, BTS], BF16, tag="ohte")
            nc.vector.tensor_scalar(ohte[:], relrep[:], iotac[:], None,
                                    ALU.is_equal)
            zoh = mp.tile([128, TPB * CH, 128], BF16, tag="zoh")
            iob = bass.AP(tensor=iotab[:].tensor, offset=iotab[:].offset,
                          ap=[iotab[:].ap[0], [0, TPB * CH], iotab[:].ap[1]])
            rslab = relb[:, i0 // 128:(i0 + BTS) // 128]
            rbc = bass.AP(tensor=rslab.tensor, offset=rslab.offset,
                          ap=[rslab.ap[0], [rslab.ap[-1][0], TPB * CH], [0, 128]])
            nc.vector.tensor_tensor(zoh[:], iob, rbc, ALU.is_equal)

            psB = ppb.tile([65, 128], F32, tag="psB")
            # per-block y_e injection matrix A[t, h] = ye_blk^T @ W1_y
            psA = ppA.tile([128, 256], F32, tag="psA")
            nc.tensor.matmul(psA[:], ye_bf[:, b * 128:(b + 1) * 128],
                             W1y_bf[:])
            ATb = sp.tile([128, 2, 128], BF16, tag="ATb")
            nc.vector.tensor_copy(ATb[:], psA[:].rearrange("p (h c) -> p h c", h=2))
            # ---- pass 1: scores s/8 per edge for the whole block ----
            scb = sp.tile([128, TPB * CH], F32, tag="scb")
            for t in range(TPB):
                e0 = t * TS
                psS = ppS.tile([128, CH, 128], F32, tag="psS")
                for c in range(CH):
                    nc.tensor.matmul(
                        psS[:, c, :],
                        gb[64:68, 0, e0 + c * 128:e0 + (c + 1) * 128],
                        V68[64:68, b * 128:(b + 1) * 128])
                sel = sp.tile([128, CH, 128], F32, tag="sel")
                nc.vector.tensor_tensor(sel[:], psS[:],
                                        zoh[:, t * CH:(t + 1) * CH, :],
                                        ALU.mult)
                nc.vector.tensor_reduce(scb[:, t * CH:(t + 1) * CH], sel[:],
                                        AX.X, ALU.add)
            # z = exp(s) via 4th-order Taylor on DVE: |s| < ~0.2 here, so
            # truncation error < 1e-5 relative and no ACT exp-table switch.
            x1 = sp.tile([128, TPB * CH], F32, tag="zx1")
            nc.vector.tensor_scalar(x1[:], scb[:], 1.0 / 4.0, 1.0, ALU.mult,
                                    ALU.add)                       # 1+x/4
            x2 = sp.tile([128, TPB * CH], F32, tag="zx2")
            nc.vector.scalar_tensor_tensor(x2[:], scb[:], 1.0 / 3.0, x1[:],
                                           ALU.mult, ALU.mult)     # x/3*(..)
            nc.vector.tensor_scalar_add(x2[:], x2[:], 1.0)         # 1+..
            x3 = sp.tile([128, TPB * CH], F32, tag="zx3")
            nc.vector.scalar_tensor_tensor(x3[:], scb[:], 1.0 / 2.0, x2[:],
                                           ALU.mult, ALU.mult)
            nc.vector.tensor_scalar_add(x3[:], x3[:], 1.0)
            zcb = sp.tile([128, TPB * CH], F32, tag="zcb")
            nc.vector.scalar_tensor_tensor(zcb[:], scb[:], 1.0, x3[:],
                                           ALU.mult, ALU.mult)
            nc.vector.tensor_scalar_add(zcb[:], zcb[:], 1.0)

            # ---- pass 2: MLP + scatter per tile ----
            for t in range(TPB):
                e0 = t * TS
                esl = slice(e0, e0 + TS)
                rhs = gb[:, 0, esl]
                ps1 = pp1.tile([128, 2, TS], F32, tag="ps1")
                nc.tensor.matmul(ps1[:, 0, :], W1a_bf[:], rhs,
                                 start=True, stop=False)
                nc.tensor.matmul(ps1[:, 0, :], ATb[:, 0, :], ohte[:, esl],
                                 start=False, stop=True)
                nc.tensor.matmul(ps1[:, 1, :], W1b_bf[:], rhs,
                                 start=True, stop=False)
                nc.tensor.matmul(ps1[:, 1, :], ATb[:, 1, :], ohte[:, esl],
                                 start=False, stop=True)
                h1 = hp.tile([128, 2, TS], BF16, tag="h1")
                nc.scalar.activation(h1[:, 0, :], ps1[:, 0, :], AF.Gelu_apprx_tanh, bias=b1a[:])
                nc.scalar.activation(h1[:, 1, :], ps1[:, 1, :], AF.Gelu_apprx_tanh, bias=b1b[:])
                ps2_ = pp2.tile([128, TS], F32, tag="ps2")
                nc.tensor.matmul(ps2_[:], W2a_bf[:], h1[:, 0, :], start=True, stop=False)
                nc.tensor.matmul(ps2_[:], W2b_bf[:], h1[:, 1, :], start=False, stop=True)
                h2 = hp.tile([128, TS], BF16, tag="h2")
                nc.scalar.activation(h2[:], ps2_[:], AF.Gelu_apprx_tanh, bias=b2c[:])
                ps3_ = pp3.tile([64, TS], F32, tag="ps3")
                nc.tensor.matmul(ps3_[:], W3_bf[:], h2[:])
                agg = sp.tile([64, TS], BF16, tag="agg")
                nc.vector.scalar_tensor_tensor(agg[:], ps3_[:], b3c[:],
                                               gb[0:64, 0, esl],
                                               ALU.add, ALU.mult)
                # scatter: per-chunk transpose + one-hot matmul accumulate
                stat = sp.tile([128, CH, 65], BF16, tag="stat")
                nc.vector.tensor_copy(stat[:, :, 64], zcb[:, t * CH:(t + 1) * CH])
                psT4 = ppt.tile([128, CH, 64], BF16, tag="psT4")
                for c in range(CH):
                    nc.tensor.transpose(psT4[:, c, :], agg[:, c * 128:(c + 1) * 128],
                                        id_bf[0:64, 0:64])
                zslab = zcb[:, t * CH:(t + 1) * CH]
                zbc = bass.AP(tensor=zslab.tensor, offset=zslab.offset,
                              ap=[zslab.ap[0], [zslab.ap[-1][0], CH], [0, 64]])
                nc.vector.tensor_tensor(stat[:, :, 0:64], psT4[:], zbc, ALU.mult)
                for c in range(CH):
                    nc.tensor.matmul(psB[:], stat[:, c, :],
                                     zoh[:, t * CH + c, :],
                                     start=(t == 0 and c == 0),
                                     stop=(t == TPB - 1 and c == CH - 1))
            # block epilogue
            sB = sp.tile([65, 128], F32, tag="sB")
            nc.vector.tensor_copy(sB[:], psB[:])
            # guard empty (padding) targets against 0/0
            nc.vector.tensor_scalar_add(sB[64:65, :], sB[64:65, :], 1e-30)
            psT2 = ppt.tile([128, 65], F32, tag="psT4")
            nc.tensor.transpose(psT2[:], sB[:], ident[0:65, 0:65])
            rec = sp.tile([128, 1], F32, tag="rec")
            nc.vector.reciprocal(rec[:], psT2[:, 64:65])
            ob = op_.tile([128, 64], BF16, tag="ob")
            nc.vector.tensor_scalar_mul(ob[:], psT2[:, 0:64], rec[:])
            nc.sync.dma_start(out_ap[b * 128:(b + 1) * 128, :], ob[:])

    resid.release()
    const.release()


def build_nc(cfg, debug=False):
    nc = bacc.Bacc("TRN2", target_bir_lowering=False, debug=debug,
                   num_devices=cfg["NC"])
    TPAD = cfg["TPAD"]
    lay32, lay16 = input_layouts(cfg)

    def blob_len(layout):
        n = 0
        for _, shape in layout:
            k = int(np.prod(shape))
            n += k + ((-k) % 64)
        return n

    cblob = nc.dram_tensor("cblob", [1, blob_len(lay32)], F32,
                           kind="ExternalInput").ap()
    dblob = nc.dram_tensor("dblob", [1, blob_len(lay16)], BF16,
                           kind="ExternalInput").ap()
    ins, _ = _carve_blob(cblob, lay32)
    ins16, _ = _carve_blob(dblob, lay16)
    ins16["srcw16"] = ins16["srcw16"].bitcast(I16)
    ins.update(ins16)
    out = nc.dram_tensor("out", [TPAD, 64], BF16, kind="ExternalOutput").ap()
    with tile.TileContext(nc) as tc:
        build(tc, out, ins, cfg)
    nc.compile()
    return nc


def _tpb_for(trg, cfg):
    """Max 512-edge tiles needed by any (core, 128-target block)."""
    tpb = 0
    for c in range(cfg["NC"]):
        t0 = c * cfg["TPC"]
        bb = np.searchsorted(trg, np.arange(t0, t0 + cfg["TPAD"] + 1, 128))
        tpb = max(tpb, int(np.ceil(np.diff(bb) / cfg["TS"]).max()))
    return max(tpb, 1)


def kernel(**inputs):
    trg = np.asarray(inputs["trg_idx"], np.int64)
    tpb = _tpb_for(trg, CFG_FULL)
    cfg = make_cfg(tpb=max(tpb, 7)) if tpb > 7 else CFG_FULL
    in_maps = host_prep(inputs, cfg)
    nc = build_nc(cfg)
    res = run_bass_kernel_spmd(nc, in_maps, core_ids=list(range(cfg["NC"])))
    outs = [np.asarray(res.results[c]["out"][:cfg["TPC"]], np.float32)
            for c in range(cfg["NC"])]
    return np.concatenate(outs, axis=0)

